# revision 38
# baseline (speedup 1.0000x reference)
"""CIN block kernel for Trainium2 (8 NeuronCores, data-parallel over batch).

Reference computation (per layer l, h0 = feat):
    out_l[b,k,d] = relu( sum_{i,j} W_l[k,i,j] * h_l[b,i,d] * feat[b,j,d] + b_l[k] )
    h_{l+1} = out_l[:, :K/2, :]   (split-half, except last layer)
    result  = concat([out0[:,128:], out1[:,128:], out2[:,:]], axis=1).sum(-1)

Mapping (per core, B_local=64, BD = B_local*D = 2048), mixed fp16/fp8:
    Tensors live as [channel, (b,d)] with (b,d) flattened on the free dim.
    The h-chain (layer-0 both halves, layer-1 k<128 half) runs fp16 so
    quantization error never propagates. The terminal outputs run fp8 e4m3
    DoubleRow matmuls (2 contraction rows/partition, 0.5 cyc/row):
      - layer-1 k>=128 half: 1-pass (W e4m3); its error is invisible under
        layer 2's because r1 rows are ~2.5x smaller than the global max
      - layer-2 both halves: 2-pass (W = e4m3 hi + e5m2 lo residual)
    z16 is produced in quad tiles (4 feat rows x 1024 positions) on DVE (a
    GPSIMD share for balance), converted to e4m3 on ACT/GPSIMD/DVE.
    h1/h2 drains run on DVE (tensor_scalar add-bias/max-0) so they do not
    queue behind ACT converts; r* drains stay on ACT with DVE d-reduces.
    Layer-0 z for half 0 comes from PE row-replication (s4 selection
    matmuls); half 1's replica streams from DRAM (featH) to cut PE/DVE work.
"""

import os
import sys

import numpy as np

for _p in ("/opt/trn_rl_repo", "/root/.axon_site/_ro/trn_rl_repo"):
    if os.path.isdir(_p) and _p not in sys.path:
        sys.path.insert(0, _p)

import concourse.bacc as bacc
import concourse.bass as bass
import concourse.mybir as mybir
import concourse.tile as tile
from concourse.bass_utils import run_bass_kernel_spmd

F32 = mybir.dt.float32
F16 = mybir.dt.float16
F8E4 = mybir.dt.float8e4
F8E5 = mybir.dt.float8e5
DR = mybir.MatmulPerfMode.DoubleRow
RELU = mybir.ActivationFunctionType.Relu
AXX = mybir.AxisListType.X
ADD = mybir.AluOpType.add
MAX = mybir.AluOpType.max

NCORES = 8
B, F0, D = 512, 32, 32
BL = B // NCORES          # 64 batch rows per core
BD = BL * D               # 2048 free positions per core
NT = 512                  # free-dim tile (one PSUM bank)
HB = 1024                 # half of BD
K = 256                   # channels per layer
H = 128                   # hidden rows fed to layers 1,2 (split-half of 256)
NJ0 = F0 * F0 // 128      # 8 partition-chunks for layer-0 (i,j) pairs
NG = F0 // 2              # 16 DoubleRow groups (j-pairs) per layer
NQ = F0 // 4              # 8 quad tiles (4 j's) per (layer, half)
QF8_L1 = 5                # per half: last QF8_L1 quads of L1-kh1 are fp8 DR

_CACHE = {}
LAST_RESULTS = None


def _build_program(
    gp_zq=((7,), (7,)),       # (L1, L2): quads whose z-mul runs on gpsimd (emitted first)
    cvt_l1=None,              # quad -> engine for L1 fp8 quads
    cvt_l2=None,              # quad -> engine for L2 quads
    qf8_l1=QF8_L1,            # per half: last qf8_l1 quads of L1-kh1 are fp8 DR
    fb_engines=("sync",),
    ps_bufs=8,
    z16_bufs=3,
    z8_bufs=8,
    dp_bufs=5,
    fb_splits=2,
    warmup_mms=8,
    w_dma_eng="scalar",
    out_dma_split=True,
    h_drain_dve=True,
    feath_dma_half1=False,
):
    if cvt_l1 is None:
        cvt_l1 = {3: "act", 4: "dve", 5: "gp", 6: "act", 7: "dve"}
    if cvt_l2 is None:
        # "dved"/"gpd": direct fp8 mul on DVE/GPSIMD (no z16, no convert)
        cvt_l2 = {0: "act", 1: "act", 2: "gpd", 3: "act",
                  4: "gpd", 5: "act", 6: "dved", 7: "dved"}

    def cvt_plan(lyr, q):
        return (cvt_l1 if lyr == 1 else cvt_l2).get(q, "act")

    def gp_zmul(lyr, q):
        return q in gp_zq[lyr - 1]

    nc = bacc.Bacc("TRN2", target_bir_lowering=False, debug=False)

    featT_d = nc.dram_tensor("featT16", [F0, BD], F16, kind="ExternalInput").ap()
    featR_d = nc.dram_tensor("featR", [128, BD], F16, kind="ExternalInput").ap()
    featH_d = nc.dram_tensor("featH", [128, NJ0 * BD], F16, kind="ExternalInput").ap()
    s4_d = nc.dram_tensor("s4all", [F0, NJ0 * 128], F16, kind="ExternalInput").ap()
    w0_d = nc.dram_tensor("w0t", [128, NJ0 * K], F16, kind="ExternalInput").ap()
    w1h_d = nc.dram_tensor("w1h16", [128, F0 * 128], F16, kind="ExternalInput").ap()
    n16_l1 = 4 * (NQ - qf8_l1)  # j's covered by fp16 kh1 quads
    w1r16_d = nc.dram_tensor(
        "w1r16", [128, max(n16_l1, 1) * 128], F16, kind="ExternalInput"
    ).ap()
    w1r_d = nc.dram_tensor("w1r8", [128, NG * 2 * 128], F8E4, kind="ExternalInput").ap()
    w2hi_d = nc.dram_tensor("w2hi8", [128, NG * 2 * 2 * 128], F8E4, kind="ExternalInput").ap()
    w2lo_d = nc.dram_tensor("w2lo8", [128, NG * 2 * 2 * 128], F8E5, kind="ExternalInput").ap()
    b0_d = nc.dram_tensor("b0t", [128, 2], F32, kind="ExternalInput").ap()
    b1_d = nc.dram_tensor("b1t", [128, 2], F32, kind="ExternalInput").ap()
    b2_d = nc.dram_tensor("b2t", [128, 2], F32, kind="ExternalInput").ap()
    out_d = nc.dram_tensor("out", [512, BL], F32, kind="ExternalOutput").ap()

    with tile.TileContext(nc) as tc:
        with (
            tc.tile_pool(name="const", bufs=1) as const,
            tc.tile_pool(name="ps", bufs=ps_bufs, space="PSUM") as ps,
            tc.tile_pool(name="z16p", bufs=z16_bufs) as z16p,
            tc.tile_pool(name="z16gp", bufs=1) as z16gp,
            tc.tile_pool(name="z8p", bufs=z8_bufs) as z8p,
            tc.tile_pool(name="zp0", bufs=3) as zp0,
            tc.tile_pool(name="dp", bufs=dp_bufs) as dp,
        ):
            if warmup_mms:
                wt = const.tile([128, NT], F16, name="warm_sb")
                nc.vector.memset(wt, 0.0)
                wps = ps.tile([128, NT], F32, tag="ps", name="warm_ps")
                for _ in range(warmup_mms):
                    nc.tensor.matmul(wps, wt[:, :128], wt, start=True, stop=True)

            # ---- L0 constants first: L0 is the front of the schedule ----
            feat16 = const.tile([F0, BD], F16)
            nc.sync.dma_start(feat16, featT_d)
            s4 = const.tile([F0, NJ0 * 128], F16)
            nc.sync.dma_start(s4, s4_d)
            featR = const.tile([128, BD], F16)
            nc.sync.dma_start(featR, featR_d)
            w0 = const.tile([128, NJ0 * K], F16)
            nc.sync.dma_start(w0, w0_d)
            b0 = const.tile([128, 2], F32)
            b1 = const.tile([128, 2], F32)
            b2 = const.tile([128, 2], F32)

            wq = getattr(nc, w_dma_eng)
            if feath_dma_half1:
                # only half-1's slice (t = 2, 3) streams from DRAM; 4 chunks
                # on the sync queue so it lands after the L0 consts and
                # before fb(0), clearing DMA_ENGINES for the front.
                featH = const.tile([128, NJ0 * BD // 2], F16, name="featH")
                hw = NJ0 * NT // 2
                for c4 in range(4):
                    nc.sync.dma_start(
                        featH[:, c4 * hw : (c4 + 1) * hw],
                        featH_d[:, 2 * NJ0 * NT + c4 * hw : 2 * NJ0 * NT + (c4 + 1) * hw],
                    )
            wq.dma_start(b0, b0_d)
            wq.dma_start(b1, b1_d)
            wq.dma_start(b2, b2_d)

            h1 = const.tile([128, BD], F16)
            h2 = const.tile([128, BD], F16)
            fb_grp = F0 // 2
            fbh = [
                const.tile([128, fb_grp * HB], F16, name=f"fbh{i}")
                for i in range(fb_splits)
            ]

            def fb_buf(half, j):
                return fbh[(2 * half + j // fb_grp) % fb_splits]

            r0 = const.tile([128, BL], F32)
            r1 = const.tile([128, BL], F32)
            r2a = const.tile([128, BL], F32)
            r2b = const.tile([128, BL], F32)

            def emit_fb(half, js=range(F0)):
                hoff = half * HB
                for j in js:
                    dst = fb_buf(half, j)[:, (j % fb_grp) * HB : (j % fb_grp + 1) * HB]
                    eng = getattr(nc, fb_engines[j % len(fb_engines)])
                    eng.dma_start(
                        dst,
                        featT_d[j : j + 1, hoff : hoff + HB].to_broadcast([128, HB]),
                    )

            def drain_h(o_ps, bias_ap, t, h_out):
                dst = h_out[:, t * NT : (t + 1) * NT]
                if h_drain_dve:
                    nc.vector.tensor_scalar(dst, o_ps, bias_ap, 0.0, op0=ADD, op1=MAX)
                else:
                    nc.scalar.activation(dst, o_ps, RELU, bias=bias_ap)

            pending_red = []

            def drain_r(o_ps, bias_ap, t, r_out):
                dx = dp.tile([128, NT], F16, tag="d", name=f"d_{t}")
                nc.scalar.activation(dx, o_ps, RELU, bias=bias_ap)
                pending_red.append((r_out, t, dx))

            def red_flush():
                # d-reduces are deferred to a point where their dx inputs are
                # already materialized, so they never head-of-line-block the
                # DVE queue in front of z production.
                while pending_red:
                    r_out, t, dx = pending_red.pop(0)
                    nc.vector.reduce_sum(
                        r_out[:, t * (NT // D) : (t + 1) * (NT // D)],
                        dx.rearrange("p (b d) -> p b d", d=D),
                        axis=AXX,
                    )

            def emit_l0(half):
                # ---------------- Layer 0 (fp16, h = feat) ----------------
                for t in (2 * half, 2 * half + 1):
                    o0 = [
                        ps.tile([128, NT], F32, tag="ps", name=f"o0_{t}_{kh}")
                        for kh in range(2)
                    ]
                    for c in range(NJ0):
                        z0 = zp0.tile([128, NT], F16, tag="z0")
                        if feath_dma_half1 and t >= 2:
                            fo = ((t - 2) * NJ0 + c) * NT
                            nc.vector.tensor_mul(
                                z0,
                                featH[:, fo : fo + NT],
                                featR[:, t * NT : (t + 1) * NT],
                            )
                        else:
                            hr_ps = ps.tile([128, NT], F32, tag="ps", name=f"hr_{t}_{c}")
                            nc.tensor.matmul(
                                hr_ps,
                                s4[:, c * 128 : (c + 1) * 128],
                                feat16[:, t * NT : (t + 1) * NT],
                                start=True,
                                stop=True,
                            )
                            nc.vector.tensor_mul(
                                z0, hr_ps, featR[:, t * NT : (t + 1) * NT]
                            )
                        for kh in range(2):
                            nc.tensor.matmul(
                                o0[kh],
                                w0[:, c * K + kh * 128 : c * K + (kh + 1) * 128],
                                z0,
                                start=(c == 0),
                                stop=(c == NJ0 - 1),
                            )
                    drain_h(o0[0], b0[:, 0:1], t, h1)
                    drain_r(o0[1], b0[:, 1:2], t, r0)

            def emit_zmul(lyr, half, ht, q, gp=False):
                """single broadcast-AP mul producing a [128, 4*HB] z16 quad."""
                if gp:
                    z16 = z16gp.tile([128, 4 * HB], F16, tag="z16g", name="z16g")
                else:
                    z16 = z16p.tile([128, 4 * HB], F16, tag="z16", name="z16")
                fb = fb_buf(half, 4 * q)
                fs = (4 * q) % fb_grp * HB
                eng = nc.gpsimd if gp else nc.vector
                hb4 = ht.rearrange("p (one n) -> p one n", one=1).to_broadcast(
                    [128, 4, HB]
                )
                eng.tensor_mul(
                    z16.rearrange("p (x n) -> p x n", x=4),
                    hb4,
                    fb[:, fs : fs + 4 * HB].rearrange("p (x n) -> p x n", x=4),
                )
                return z16

            def emit_cvt(lyr, half, q, z16, z8m):
                z8 = z8p.tile(
                    [128, 4 * HB], F8E4, tag="z8", name=f"z8_{lyr}_{half}_{q}"
                )
                ce = cvt_plan(lyr, q)
                if ce == "act":
                    nc.scalar.copy(z8, z16)
                elif ce == "gp":
                    nc.gpsimd.tensor_copy(z8, z16)
                else:
                    nc.vector.tensor_scalar_mul(z8, z16, 1.0)
                z8m[q] = z8

            def dr_mm(o_u, w8, zv, a, u, g, glast, passes_done):
                zu = zv[:, 2 * a : 2 * a + 2, u * NT : (u + 1) * NT]
                nc.tensor.matmul(
                    o_u,
                    w8.rearrange("p (two m) -> p two m", two=2),
                    zu,
                    start=(g == 0 and not passes_done),
                    stop=glast,
                    perf_mode=DR,
                    skip_group_check=True,
                )

            def emit_l1(half, z8m):
                # z production + kh0 fp16 matmuls + kh1 (fp16 quads inline,
                # fp8 DR quads after). GP z-quads are emitted first so the
                # slow engine works ahead of its consumption point.
                hoff = half * HB
                ht = h1[:, hoff : hoff + HB]
                o = [
                    ps.tile([128, NT], F32, tag="ps", name=f"o1h_{half}_{u}")
                    for u in range(2)
                ]
                orr = [
                    ps.tile([128, NT], F32, tag="ps", name=f"o1r_{half}_{u}")
                    for u in range(2)
                ]
                zq = {}
                for q in range(NQ):
                    if gp_zmul(1, q):
                        zq[q] = emit_zmul(1, half, ht, q, gp=True)
                        if q >= NQ - qf8_l1:
                            emit_cvt(1, half, q, zq[q], z8m)
                for q in range(NQ):
                    if q == 3:
                        red_flush()   # r0 / previous half's r2 reduces
                    z16 = zq.get(q)
                    if z16 is None:
                        z16 = emit_zmul(1, half, ht, q)
                        if q >= NQ - qf8_l1:
                            emit_cvt(1, half, q, z16, z8m)
                    fp16_kh1 = q < NQ - qf8_l1
                    for jj in range(4):
                        j = 4 * q + jj
                        for u in range(2):
                            nc.tensor.matmul(
                                o[u],
                                w1h[:, j * 128 : (j + 1) * 128],
                                z16[:, jj * HB + u * NT : jj * HB + (u + 1) * NT],
                                start=(q == 0 and jj == 0),
                                stop=(q == NQ - 1 and jj == 3),
                            )
                            if fp16_kh1:
                                nc.tensor.matmul(
                                    orr[u],
                                    w1r16[:, j * 128 : (j + 1) * 128],
                                    z16[:, jj * HB + u * NT : jj * HB + (u + 1) * NT],
                                    start=(q == 0 and jj == 0),
                                    stop=(qf8_l1 == 0 and q == NQ - 1 and jj == 3),
                                    skip_group_check=True,
                                )
                return o, orr

            def emit_l1_dr(half, orr, z8m):
                # fp8 DR quads for kh1
                for q in range(NQ - qf8_l1, NQ):
                    zv = z8m[q].rearrange("p (four n) -> p four n", four=4)
                    for a in range(2):
                        g = 2 * q + a
                        wv = w1r[:, g * 256 : (g + 1) * 256]
                        for u in range(2):
                            dr_mm(orr[u], wv, zv, a, u, g,
                                  glast=(g == NG - 1),
                                  passes_done=(qf8_l1 < NQ))

            def emit_zmul8(half, ht, q, gp=False):
                """direct fp8 e4m3 product (no z16 stage)."""
                z8 = z8p.tile([128, 4 * HB], F8E4, tag="z8", name=f"z8d_{half}_{q}")
                fb = fb_buf(half, 4 * q)
                fs = (4 * q) % fb_grp * HB
                eng = nc.gpsimd if gp else nc.vector
                hb4 = ht.rearrange("p (one n) -> p one n", one=1).to_broadcast(
                    [128, 4, HB]
                )
                eng.tensor_mul(
                    z8.rearrange("p (x n) -> p x n", x=4),
                    hb4,
                    fb[:, fs : fs + 4 * HB].rearrange("p (x n) -> p x n", x=4),
                )
                return z8

            L2_PROD = [0, 6, 1, 7, 3, 5]      # non-gp quads, z16/direct mix
            L2_MM = [0, 2, 6, 1, 4, 7, 3, 5]  # approx completion order

            def l2_qorder():
                gp_first = [q for q in range(NQ) if cvt_plan(2, q) == "gpd"]
                rest = [q for q in L2_PROD if q not in gp_first]
                extra = [q for q in range(NQ) if q not in gp_first + rest]
                return gp_first + rest + extra

            def emit_l2_z(half, z8m):
                hoff = half * HB
                ht = h2[:, hoff : hoff + HB]
                for q in l2_qorder():
                    ce = cvt_plan(2, q)
                    if ce == "gpd":
                        z8m[q] = emit_zmul8(half, ht, q, gp=True)
                    elif ce == "dved":
                        z8m[q] = emit_zmul8(half, ht, q)
                    else:
                        emit_cvt(2, half, q, emit_zmul(2, half, ht, q), z8m)

            def emit_l2_mms(half, z8m):
                o = [
                    [
                        ps.tile([128, NT], F32, tag="ps", name=f"o2_{half}_{kh}_{u}")
                        for u in range(2)
                    ]
                    for kh in range(2)
                ]
                order = [q for q in L2_MM if q in z8m] + [
                    q for q in range(NQ) if q not in L2_MM and q in z8m
                ]
                for qi, q in enumerate(order):
                    zv = z8m[q].rearrange("p (four n) -> p four n", four=4)
                    first_q = qi == 0
                    last_q = qi == NQ - 1
                    for a in range(2):
                        g = 2 * q + a
                        for kh in range(2):
                            base = (g * 2 + kh) * 256
                            for u in range(2):
                                zu = zv[:, 2 * a : 2 * a + 2, u * NT : (u + 1) * NT]
                                nc.tensor.matmul(
                                    o[kh][u],
                                    w2hi[:, base : base + 256].rearrange(
                                        "p (two m) -> p two m", two=2
                                    ),
                                    zu,
                                    start=(first_q and a == 0), stop=False,
                                    perf_mode=DR, skip_group_check=True,
                                )
                                nc.tensor.matmul(
                                    o[kh][u],
                                    w2lo[:, base : base + 256].rearrange(
                                        "p (two m) -> p two m", two=2
                                    ),
                                    zu,
                                    start=False, stop=(last_q and a == 1),
                                    perf_mode=DR, skip_group_check=True,
                                )
                return o

            def emit_out(half):
                cs = slice(half * BL // 2, (half + 1) * BL // 2)
                nc.sync.dma_start(out_d[0:128, cs], r0[:, cs])
                nc.sync.dma_start(out_d[128:256, cs], r1[:, cs])
                nc.sync.dma_start(out_d[256:384, cs], r2a[:, cs])
                nc.sync.dma_start(out_d[384:512, cs], r2b[:, cs])

            # ---------------- emission schedule ----------------
            emit_fb(0)
            # weight loads for layers 1/2 on a separate queue after fb(0)
            w1h = const.tile([128, F0 * 128], F16)
            wq.dma_start(w1h, w1h_d)
            w1r16 = const.tile([128, max(n16_l1, 1) * 128], F16)
            if n16_l1:
                wq.dma_start(w1r16, w1r16_d)
            w1r = const.tile([128, NG * 2 * 128], F8E4)
            wq.dma_start(w1r, w1r_d)
            w2hi = const.tile([128, NG * 2 * 2 * 128], F8E4)
            wq.dma_start(w2hi, w2hi_d)
            w2lo = const.tile([128, NG * 2 * 2 * 128], F8E5)
            wq.dma_start(w2lo, w2lo_d)

            # Half-0's L1 A-phase runs right after L0(0); L0(1)'s hr/o0
            # matmuls then fill the PE while half-0's converts complete and
            # L2(0)'s z pipeline spins up behind the h2(0) drains.
            emit_l0(0)
            z8m_1 = {}
            o1h, o1r = emit_l1(0, z8m_1)
            for u in range(2):
                drain_h(o1h[u], b1[:, 0:1], u, h2)
            emit_l0(1)
            emit_l1_dr(0, o1r, z8m_1)
            for u in range(2):
                drain_r(o1r[u], b1[:, 1:2], u, r1)

            z8m2 = {}
            emit_l2_z(0, z8m2)
            red_flush()   # r0 t2/t3 + r1(0) reduces
            # fb(1) reuses the fbh buffers: emit only after every half-0
            # reader (incl. L2's z muls) is in the graph.
            emit_fb(1)
            o2 = emit_l2_mms(0, z8m2)
            for u in range(2):
                drain_r(o2[0][u], b2[:, 0:1], u, r2a)
                drain_r(o2[1][u], b2[:, 1:2], u, r2b)

            # half 1 (emit_l1's q3 flush emits half-0's r2 reduces; out(0)
            # follows so it reads fully-reduced tiles)
            z8m_1 = {}
            o1h, o1r = emit_l1(1, z8m_1)
            if out_dma_split:
                emit_out(0)
            for u in range(2):
                drain_h(o1h[u], b1[:, 0:1], 2 + u, h2)
            emit_l1_dr(1, o1r, z8m_1)
            for u in range(2):
                drain_r(o1r[u], b1[:, 1:2], 2 + u, r1)

            z8m2 = {}
            emit_l2_z(1, z8m2)
            red_flush()
            o2 = emit_l2_mms(1, z8m2)
            for u in range(2):
                drain_r(o2[0][u], b2[:, 0:1], 2 + u, r2a)
                drain_r(o2[1][u], b2[:, 1:2], 2 + u, r2b)
            red_flush()
            if out_dma_split:
                emit_out(1)

            if not out_dma_split:
                nc.sync.dma_start(out_d[0:128, :], r0)
                nc.sync.dma_start(out_d[128:256, :], r1)
                nc.sync.dma_start(out_d[256:384, :], r2a)
                nc.sync.dma_start(out_d[384:512, :], r2b)

    nc.compile()
    return nc


def _host_prep(feat, W0, b0, W1, b1, W2, b2):
    """Rearrange full inputs into the per-core in_maps."""
    import ml_dtypes

    E4 = ml_dtypes.float8_e4m3fn
    E5 = ml_dtypes.float8_e5m2
    feat = np.ascontiguousarray(feat, dtype=np.float32)

    # W0: chunks c of 128 (i,j)-pairs, i-major: p = (i_local, j), i = 4c + p//32
    A = np.ascontiguousarray(W0.transpose(1, 2, 0)).reshape(F0 * F0, K)
    w0t = np.ascontiguousarray(
        A.reshape(NJ0, 128, K).transpose(1, 0, 2).reshape(128, NJ0 * K)
    ).astype(np.float16)

    # L1 kh0 (h-half) fp16: [i, j*128 + m] = W1[m, i, j]
    w1h16 = np.ascontiguousarray(
        W1[:128].transpose(1, 2, 0).reshape(H, F0 * 128)
    ).astype(np.float16)
    # L1 kh1 fp8 1-pass: [i, (g*2+pair)*128 + m] = q8(W1[128+m, i, 2g+pair])
    w1r = W1[128:].transpose(1, 2, 0)            # [i, j, m]
    w1r8 = np.clip(w1r, -240, 240).astype(E4).reshape(H, F0 * 128)
    # fp16 copy of the first j's for the fp16 kh1 quads
    n16_l1 = 4 * (NQ - QF8_L1)
    w1r16 = np.ascontiguousarray(
        w1r[:, : max(n16_l1, 1)].reshape(H, -1)
    ).astype(np.float16)

    # L2 both kh fp8 2-pass: [i, ((g*2+kh)*2+pair)*128 + m] = W2[kh*128+m, i, 2g+pair]
    w2 = W2.transpose(1, 2, 0).reshape(H, NG, 2, 2, 128)  # [i, g, pair, kh, m]
    w2 = np.ascontiguousarray(w2.transpose(0, 1, 3, 2, 4))  # [i, g, kh, pair, m]
    w2hi = np.clip(w2, -240, 240).astype(E4)
    w2lo = (w2 - w2hi.astype(np.float32)).astype(E5)
    w2hi8 = w2hi.reshape(H, NG * 2 * 2 * 128)
    w2lo8 = w2lo.reshape(H, NG * 2 * 2 * 128)

    p_ = np.arange(128)
    s4all = np.zeros((F0, NJ0 * 128), np.float16)
    for cc in range(NJ0):
        s4all[:, cc * 128 : (cc + 1) * 128] = (
            (4 * cc + p_[None, :] // F0) == np.arange(F0)[:, None]
        )

    b0t = np.ascontiguousarray(b0.reshape(2, 128).T).astype(np.float32)
    b1t = np.ascontiguousarray(b1.reshape(2, 128).T).astype(np.float32)
    b2t = np.ascontiguousarray(b2.reshape(2, 128).T).astype(np.float32)

    p = np.arange(128)
    in_maps = []
    for c in range(NCORES):
        fc = feat[c * BL : (c + 1) * BL]                        # [64, 32, 32]
        featT = np.ascontiguousarray(fc.transpose(1, 0, 2)).reshape(F0, BD)
        featT = featT.astype(np.float16)
        featR = np.ascontiguousarray(featT[p % F0])             # [128, BD]
        featH = np.concatenate(
            [
                featT[4 * cc + p // F0, t * NT : (t + 1) * NT]
                for t in range(BD // NT)
                for cc in range(NJ0)
            ],
            axis=1,
        )                                                        # [128, NJ0*BD] t-major
        in_maps.append(
            {
                "featT16": featT,
                "featR": featR,
                "featH": np.ascontiguousarray(featH),
                "s4all": s4all,
                "w0t": w0t,
                "w1h16": w1h16,
                "w1r16": w1r16,
                "w1r8": w1r8.view(np.uint8),
                "w2hi8": w2hi8.view(np.uint8),
                "w2lo8": w2lo8.view(np.uint8),
                "b0t": b0t,
                "b1t": b1t,
                "b2t": b2t,
            }
        )
    return in_maps


def kernel(feat, W0, b0, W1, b1, W2, b2):
    global LAST_RESULTS
    if "nc" not in _CACHE:
        _CACHE["nc"] = _build_program()
    nc = _CACHE["nc"]
    in_maps = _host_prep(feat, W0, b0, W1, b1, W2, b2)
    res = run_bass_kernel_spmd(nc, in_maps, core_ids=list(range(NCORES)))
    LAST_RESULTS = res
    out = np.concatenate([res.results[c]["out"].T for c in range(NCORES)], axis=0)
    return np.ascontiguousarray(out, dtype=np.float32)


# revision 48
# speedup vs baseline: 1.2674x; 1.2674x over previous
"""CIN block kernel for Trainium2 (8 NeuronCores, data-parallel over batch).

Reference computation (per layer l, h0 = feat):
    out_l[b,k,d] = relu( sum_{i,j} W_l[k,i,j] * h_l[b,i,d] * feat[b,j,d] + b_l[k] )
    h_{l+1} = out_l[:, :K/2, :]   (split-half, except last layer)
    result  = concat([out0[:,128:], out1[:,128:], out2[:,:]], axis=1).sum(-1)

Mapping (per core, B_local=64, BD = B_local*D = 2048), mixed fp16/fp8:
    Tensors live as [channel, (b,d)] with (b,d) flattened on the free dim.
    The h-chain (layer-0 both halves, layer-1 k<128 half) runs fp16 so
    quantization error never propagates. The terminal outputs run fp8 e4m3
    DoubleRow matmuls (2 contraction rows/partition, 0.5 cyc/row):
      - layer-1 k>=128 half: 1-pass (W e4m3); its error is invisible under
        layer 2's because r1 rows are ~2.5x smaller than the global max
      - layer-2 both halves: 2-pass (W = e4m3 hi + e5m2 lo residual)
    z16 is produced in quad tiles (4 feat rows x 1024 positions) on DVE (a
    GPSIMD share for balance), converted to e4m3 on ACT/GPSIMD/DVE.
    h1/h2 drains run on DVE (tensor_scalar add-bias/max-0) so they do not
    queue behind ACT converts; r* drains stay on ACT with DVE d-reduces.
    Layer-0 z for half 0 comes from PE row-replication (s4 selection
    matmuls); half 1's replica streams from DRAM (featH) to cut PE/DVE work.
"""

import os
import sys

import numpy as np

for _p in ("/opt/trn_rl_repo", "/root/.axon_site/_ro/trn_rl_repo"):
    if os.path.isdir(_p) and _p not in sys.path:
        sys.path.insert(0, _p)

import concourse.bacc as bacc
import concourse.bass as bass
import concourse.mybir as mybir
import concourse.tile as tile
from concourse.bass_utils import run_bass_kernel_spmd

F32 = mybir.dt.float32
F16 = mybir.dt.float16
F8E4 = mybir.dt.float8e4
F8E5 = mybir.dt.float8e5
DR = mybir.MatmulPerfMode.DoubleRow
RELU = mybir.ActivationFunctionType.Relu
AXX = mybir.AxisListType.X
ADD = mybir.AluOpType.add
MAX = mybir.AluOpType.max

NCORES = 8
B, F0, D = 512, 32, 32
BL = B // NCORES          # 64 batch rows per core
BD = BL * D               # 2048 free positions per core
NT = 512                  # free-dim tile (one PSUM bank)
HB = 1024                 # half of BD
K = 256                   # channels per layer
H = 128                   # hidden rows fed to layers 1,2 (split-half of 256)
NJ0 = F0 * F0 // 128      # 8 partition-chunks for layer-0 (i,j) pairs
NG = F0 // 2              # 16 DoubleRow groups (j-pairs) per layer
NQ = F0 // 4              # 8 quad tiles (4 j's) per (layer, half)
QF8_L1 = 5                # per half: last QF8_L1 quads of L1-kh1 are fp8 DR

_CACHE = {}
LAST_RESULTS = None


def _build_program(
    gp_zq=((7,), (7,)),       # (L1, L2): quads whose z-mul runs on gpsimd (emitted first)
    cvt_l1=None,              # quad -> engine for L1 fp8 quads
    cvt_l2=None,              # quad -> engine for L2 quads
    qf8_l1=QF8_L1,            # per half: last qf8_l1 quads of L1-kh1 are fp8 DR
    fb_engines=("sync",),
    ps_bufs=8,
    z16_bufs=3,
    z8_bufs=8,
    dp_bufs=5,
    fb_splits=2,
    warmup_mms=8,
    w_dma_eng="scalar",
    out_dma_split=True,
    h_drain_dve=False,
    feath_dma_half1=False,
):
    if cvt_l1 is None:
        # half 0 has the L0(1) PE filler after it, so its DR can wait on the
        # longer ACT chain; half 1's DR is seam-critical and keeps q4 on DVE
        cvt_l1 = ({3: "act", 4: "dve", 5: "gp", 6: "act", 7: "act"},
                  {3: "act", 4: "dve", 5: "gp", 6: "act", 7: "act"})
    if cvt_l2 is None:
        # "dved"/"gpd": direct fp8 mul on DVE/GPSIMD (no z16, no convert)
        cvt_l2 = {0: "act", 1: "act", 2: "gpd", 3: "act",
                  4: "gpd", 5: "act", 6: "dved", 7: "dved"}

    def cvt_plan(lyr, q, half=0):
        if lyr == 1:
            m = cvt_l1[half] if isinstance(cvt_l1, tuple) else cvt_l1
        else:
            m = cvt_l2[half] if isinstance(cvt_l2, tuple) else cvt_l2
        return m.get(q, "act")

    def gp_zmul(lyr, q):
        return q in gp_zq[lyr - 1]

    nc = bacc.Bacc("TRN2", target_bir_lowering=False, debug=False)

    featT_d = nc.dram_tensor("featT16", [F0, BD], F16, kind="ExternalInput").ap()
    featR_d = nc.dram_tensor("featR", [128, BD], F16, kind="ExternalInput").ap()
    featH_d = nc.dram_tensor("featH", [128, NJ0 * BD], F16, kind="ExternalInput").ap()
    s4_d = nc.dram_tensor("s4all", [F0, NJ0 * 128], F16, kind="ExternalInput").ap()
    w0_d = nc.dram_tensor("w0t", [128, NJ0 * K], F16, kind="ExternalInput").ap()
    w1h_d = nc.dram_tensor("w1h16", [128, F0 * 128], F16, kind="ExternalInput").ap()
    n16_l1 = 4 * (NQ - qf8_l1)  # j's covered by fp16 kh1 quads
    w1r16_d = nc.dram_tensor(
        "w1r16", [128, max(n16_l1, 1) * 128], F16, kind="ExternalInput"
    ).ap()
    w1r_d = nc.dram_tensor("w1r8", [128, NG * 2 * 128], F8E4, kind="ExternalInput").ap()
    w2hi_d = nc.dram_tensor("w2hi8", [128, NG * 2 * 2 * 128], F8E4, kind="ExternalInput").ap()
    w2lo_d = nc.dram_tensor("w2lo8", [128, NG * 2 * 2 * 128], F8E5, kind="ExternalInput").ap()
    b0_d = nc.dram_tensor("b0t", [128, 2], F32, kind="ExternalInput").ap()
    b1_d = nc.dram_tensor("b1t", [128, 2], F32, kind="ExternalInput").ap()
    b2_d = nc.dram_tensor("b2t", [128, 2], F32, kind="ExternalInput").ap()
    out_d = nc.dram_tensor("out", [512, BL], F32, kind="ExternalOutput").ap()

    with tile.TileContext(nc) as tc:
        with (
            tc.tile_pool(name="const", bufs=1) as const,
            tc.tile_pool(name="ps", bufs=ps_bufs, space="PSUM") as ps,
            tc.tile_pool(name="z16p", bufs=z16_bufs) as z16p,
            tc.tile_pool(name="z16gp", bufs=1) as z16gp,
            tc.tile_pool(name="z8p", bufs=z8_bufs) as z8p,
            tc.tile_pool(name="zp0", bufs=3) as zp0,
            tc.tile_pool(name="dp", bufs=dp_bufs) as dp,
        ):
            if warmup_mms:
                wt = const.tile([128, NT], F16, name="warm_sb")
                nc.vector.memset(wt, 0.0)
                wps = ps.tile([128, NT], F32, tag="ps", name="warm_ps")
                for _ in range(warmup_mms):
                    nc.tensor.matmul(wps, wt[:, :128], wt, start=True, stop=True)

            # ---- L0 constants first: L0 is the front of the schedule ----
            feat16 = const.tile([F0, BD], F16)
            nc.sync.dma_start(feat16, featT_d)
            s4 = const.tile([F0, NJ0 * 128], F16)
            nc.sync.dma_start(s4, s4_d)
            featR = const.tile([128, BD], F16)
            nc.sync.dma_start(featR, featR_d)
            w0 = const.tile([128, NJ0 * K], F16)
            nc.sync.dma_start(w0, w0_d)
            b0 = const.tile([128, 2], F32)
            b1 = const.tile([128, 2], F32)
            b2 = const.tile([128, 2], F32)

            wq = getattr(nc, w_dma_eng)
            if feath_dma_half1:
                # only half-1's slice (t = 2, 3) streams from DRAM; 4 chunks
                # on the sync queue so it lands after the L0 consts and
                # before fb(0), clearing DMA_ENGINES for the front.
                featH = const.tile([128, NJ0 * BD // 2], F16, name="featH")
                hw = NJ0 * NT // 2
                for c4 in range(4):
                    nc.sync.dma_start(
                        featH[:, c4 * hw : (c4 + 1) * hw],
                        featH_d[:, 2 * NJ0 * NT + c4 * hw : 2 * NJ0 * NT + (c4 + 1) * hw],
                    )
            wq.dma_start(b0, b0_d)
            wq.dma_start(b1, b1_d)
            wq.dma_start(b2, b2_d)

            h1 = const.tile([128, BD], F16)
            h2 = const.tile([128, BD], F16)
            fb_grp = F0 // 2
            fbh = [
                const.tile([128, fb_grp * HB], F16, name=f"fbh{i}")
                for i in range(fb_splits)
            ]

            def fb_buf(half, j):
                return fbh[(2 * half + j // fb_grp) % fb_splits]

            r0 = const.tile([128, BL], F32)
            r1 = const.tile([128, BL], F32)
            r2a = const.tile([128, BL], F32)
            r2b = const.tile([128, BL], F32)

            def emit_fb(half, js=range(F0)):
                hoff = half * HB
                for j in js:
                    dst = fb_buf(half, j)[:, (j % fb_grp) * HB : (j % fb_grp + 1) * HB]
                    eng = getattr(nc, fb_engines[j % len(fb_engines)])
                    eng.dma_start(
                        dst,
                        featT_d[j : j + 1, hoff : hoff + HB].to_broadcast([128, HB]),
                    )

            def drain_h(o_ps, bias_ap, t, h_out, dve=False):
                dst = h_out[:, t * NT : (t + 1) * NT]
                if dve:
                    nc.vector.tensor_scalar(dst, o_ps, bias_ap, 0.0, op0=ADD, op1=MAX)
                else:
                    nc.scalar.activation(dst, o_ps, RELU, bias=bias_ap)

            pending_red = []

            def drain_r(o_ps, bias_ap, t, r_out, dve=False):
                dx = dp.tile([128, NT], F16, tag="d", name=f"d_{t}")
                if dve:
                    nc.vector.tensor_scalar(dx, o_ps, bias_ap, 0.0, op0=ADD, op1=MAX)
                else:
                    nc.scalar.activation(dx, o_ps, RELU, bias=bias_ap)
                pending_red.append((r_out, t, dx))

            def red_flush(k=99):
                # d-reduces are deferred to a point where their dx inputs are
                # already materialized, so they never head-of-line-block the
                # DVE queue in front of z production.
                while pending_red and k > 0:
                    k -= 1
                    r_out, t, dx = pending_red.pop(0)
                    nc.vector.reduce_sum(
                        r_out[:, t * (NT // D) : (t + 1) * (NT // D)],
                        dx.rearrange("p (b d) -> p b d", d=D),
                        axis=AXX,
                    )

            def emit_l0(half):
                # ---------------- Layer 0 (fp16, h = feat) ----------------
                for t in (2 * half, 2 * half + 1):
                    o0 = [
                        ps.tile([128, NT], F32, tag="ps", name=f"o0_{t}_{kh}")
                        for kh in range(2)
                    ]
                    for c in range(NJ0):
                        z0 = zp0.tile([128, NT], F16, tag="z0")
                        if feath_dma_half1 and t >= 2:
                            fo = ((t - 2) * NJ0 + c) * NT
                            nc.vector.tensor_mul(
                                z0,
                                featH[:, fo : fo + NT],
                                featR[:, t * NT : (t + 1) * NT],
                            )
                        else:
                            hr_ps = ps.tile([128, NT], F32, tag="ps", name=f"hr_{t}_{c}")
                            nc.tensor.matmul(
                                hr_ps,
                                s4[:, c * 128 : (c + 1) * 128],
                                feat16[:, t * NT : (t + 1) * NT],
                                start=True,
                                stop=True,
                            )
                            nc.vector.tensor_mul(
                                z0, hr_ps, featR[:, t * NT : (t + 1) * NT]
                            )
                        for kh in range(2):
                            nc.tensor.matmul(
                                o0[kh],
                                w0[:, c * K + kh * 128 : c * K + (kh + 1) * 128],
                                z0,
                                start=(c == 0),
                                stop=(c == NJ0 - 1),
                            )
                    drain_h(o0[0], b0[:, 0:1], t, h1)
                    drain_r(o0[1], b0[:, 1:2], t, r0)

            def emit_zmul(lyr, half, ht, q, gp=False):
                """single broadcast-AP mul producing a [128, 4*HB] z16 quad."""
                if gp:
                    z16 = z16gp.tile([128, 4 * HB], F16, tag="z16g", name="z16g")
                else:
                    z16 = z16p.tile([128, 4 * HB], F16, tag="z16", name="z16")
                fb = fb_buf(half, 4 * q)
                fs = (4 * q) % fb_grp * HB
                eng = nc.gpsimd if gp else nc.vector
                hb4 = ht.rearrange("p (one n) -> p one n", one=1).to_broadcast(
                    [128, 4, HB]
                )
                eng.tensor_mul(
                    z16.rearrange("p (x n) -> p x n", x=4),
                    hb4,
                    fb[:, fs : fs + 4 * HB].rearrange("p (x n) -> p x n", x=4),
                )
                return z16

            def emit_cvt(lyr, half, q, z16, z8m):
                z8 = z8p.tile(
                    [128, 4 * HB], F8E4, tag="z8", name=f"z8_{lyr}_{half}_{q}"
                )
                ce = cvt_plan(lyr, q, half)
                if ce == "act":
                    nc.scalar.copy(z8, z16)
                elif ce == "gp":
                    nc.gpsimd.tensor_copy(z8, z16)
                else:
                    nc.vector.tensor_scalar_mul(z8, z16, 1.0)
                z8m[q] = z8

            def dr_mm(o_u, w8, zv, a, u, g, glast, passes_done):
                zu = zv[:, 2 * a : 2 * a + 2, u * NT : (u + 1) * NT]
                nc.tensor.matmul(
                    o_u,
                    w8.rearrange("p (two m) -> p two m", two=2),
                    zu,
                    start=(g == 0 and not passes_done),
                    stop=glast,
                    perf_mode=DR,
                    skip_group_check=True,
                )

            def emit_l1(half, z8m):
                # z production + kh0 fp16 matmuls + kh1 (fp16 quads inline,
                # fp8 DR quads after). GP z-quads are emitted first so the
                # slow engine works ahead of its consumption point.
                hoff = half * HB
                ht = h1[:, hoff : hoff + HB]
                o = [
                    ps.tile([128, NT], F32, tag="ps", name=f"o1h_{half}_{u}")
                    for u in range(2)
                ]
                orr = [
                    ps.tile([128, NT], F32, tag="ps", name=f"o1r_{half}_{u}")
                    for u in range(2)
                ]
                zq = {}
                for q in range(NQ):
                    if gp_zmul(1, q):
                        zq[q] = emit_zmul(1, half, ht, q, gp=True)
                for q in range(NQ):
                    if q == 3 and half == 0:
                        red_flush()   # r0 t0/t1 reduces
                    z16 = zq.get(q)
                    if z16 is None:
                        z16 = emit_zmul(1, half, ht, q)
                    if q >= NQ - qf8_l1:
                        emit_cvt(1, half, q, z16, z8m)
                    fp16_kh1 = q < NQ - qf8_l1
                    for jj in range(4):
                        j = 4 * q + jj
                        for u in range(2):
                            nc.tensor.matmul(
                                o[u],
                                w1h[:, j * 128 : (j + 1) * 128],
                                z16[:, jj * HB + u * NT : jj * HB + (u + 1) * NT],
                                start=(q == 0 and jj == 0),
                                stop=(q == NQ - 1 and jj == 3),
                            )
                            if fp16_kh1:
                                nc.tensor.matmul(
                                    orr[u],
                                    w1r16[:, j * 128 : (j + 1) * 128],
                                    z16[:, jj * HB + u * NT : jj * HB + (u + 1) * NT],
                                    start=(q == 0 and jj == 0),
                                    stop=(qf8_l1 == 0 and q == NQ - 1 and jj == 3),
                                    skip_group_check=True,
                                )
                if half == 1:
                    red_flush(4)  # half-0's r2 reduces, after our z quads
                return o, orr

            def emit_l1_dr(half, orr, z8m):
                # fp8 DR quads for kh1
                for q in range(NQ - qf8_l1, NQ):
                    zv = z8m[q].rearrange("p (four n) -> p four n", four=4)
                    for a in range(2):
                        g = 2 * q + a
                        wv = w1r[:, g * 256 : (g + 1) * 256]
                        for u in range(2):
                            dr_mm(orr[u], wv, zv, a, u, g,
                                  glast=(g == NG - 1),
                                  passes_done=(qf8_l1 < NQ))

            def emit_zmul8(half, ht, q, gp=False, split=False):
                """direct fp8 e4m3 product (no z16 stage)."""
                z8 = z8p.tile([128, 4 * HB], F8E4, tag="z8", name=f"z8d_{half}_{q}")
                fb = fb_buf(half, 4 * q)
                fs = (4 * q) % fb_grp * HB
                eng = nc.gpsimd if gp else nc.vector
                nsub = 2 if split else 1
                w = 4 // nsub
                hbx = ht.rearrange("p (one n) -> p one n", one=1).to_broadcast(
                    [128, w, HB]
                )
                for ss in range(nsub):
                    eng.tensor_mul(
                        z8[:, ss * w * HB : (ss + 1) * w * HB].rearrange(
                            "p (x n) -> p x n", x=w
                        ),
                        hbx,
                        fb[:, fs + ss * w * HB : fs + (ss + 1) * w * HB].rearrange(
                            "p (x n) -> p x n", x=w
                        ),
                    )
                return z8

            L2_PROD = [6, 0, 1, 7, 3, 5]      # non-gp quads, dved first
            L2_MM = [6, 0, 2, 1, 7, 4, 3, 5]  # approx completion order

            def l2_qorder():
                gp_first = [q for q in range(NQ) if cvt_plan(2, q) == "gpd"]
                rest = [q for q in L2_PROD if q not in gp_first]
                extra = [q for q in range(NQ) if q not in gp_first + rest]
                return gp_first + rest + extra

            def emit_l2_z(half, z8m):
                hoff = half * HB
                ht = h2[:, hoff : hoff + HB]
                for q in l2_qorder():
                    ce = cvt_plan(2, q, half)
                    if ce == "gpd":
                        z8m[q] = emit_zmul8(half, ht, q, gp=True)
                    elif ce == "dved":
                        z8m[q] = emit_zmul8(half, ht, q)
                    else:
                        emit_cvt(2, half, q, emit_zmul(2, half, ht, q), z8m)

            def emit_l2_mms(half, z8m):
                o = [
                    [
                        ps.tile([128, NT], F32, tag="ps", name=f"o2_{half}_{kh}_{u}")
                        for u in range(2)
                    ]
                    for kh in range(2)
                ]
                order = [q for q in L2_MM if q in z8m] + [
                    q for q in range(NQ) if q not in L2_MM and q in z8m
                ]
                for qi, q in enumerate(order):
                    zv = z8m[q].rearrange("p (four n) -> p four n", four=4)
                    first_q = qi == 0
                    last_q = qi == NQ - 1
                    for a in range(2):
                        g = 2 * q + a
                        for kh in range(2):
                            base = (g * 2 + kh) * 256
                            for u in range(2):
                                zu = zv[:, 2 * a : 2 * a + 2, u * NT : (u + 1) * NT]
                                nc.tensor.matmul(
                                    o[kh][u],
                                    w2hi[:, base : base + 256].rearrange(
                                        "p (two m) -> p two m", two=2
                                    ),
                                    zu,
                                    start=(first_q and a == 0), stop=False,
                                    perf_mode=DR, skip_group_check=True,
                                )
                                nc.tensor.matmul(
                                    o[kh][u],
                                    w2lo[:, base : base + 256].rearrange(
                                        "p (two m) -> p two m", two=2
                                    ),
                                    zu,
                                    start=False, stop=(last_q and a == 1),
                                    perf_mode=DR, skip_group_check=True,
                                )
                return o

            def emit_out(half):
                cs = slice(half * BL // 2, (half + 1) * BL // 2)
                nc.sync.dma_start(out_d[0:128, cs], r0[:, cs])
                nc.sync.dma_start(out_d[128:256, cs], r1[:, cs])
                nc.sync.dma_start(out_d[256:384, cs], r2a[:, cs])
                nc.sync.dma_start(out_d[384:512, cs], r2b[:, cs])

            # ---------------- emission schedule ----------------
            emit_fb(0, js=[28, 29, 30, 31] + list(range(28)))
            # weight loads for layers 1/2 on a separate queue after fb(0)
            w1h = const.tile([128, F0 * 128], F16)
            wq.dma_start(w1h, w1h_d)
            w1r16 = const.tile([128, max(n16_l1, 1) * 128], F16)
            if n16_l1:
                wq.dma_start(w1r16, w1r16_d)
            w1r = const.tile([128, NG * 2 * 128], F8E4)
            nc.sync.dma_start(w1r, w1r_d)
            w2hi = const.tile([128, NG * 2 * 2 * 128], F8E4)
            nc.sync.dma_start(w2hi, w2hi_d)
            w2lo = const.tile([128, NG * 2 * 2 * 128], F8E5)
            nc.sync.dma_start(w2lo, w2lo_d)

            # Half-0's L1 A-phase runs right after L0(0); L0(1)'s hr/o0
            # matmuls then fill the PE while half-0's converts complete and
            # L2(0)'s z pipeline spins up behind the h2(0) drains.
            emit_l0(0)
            z8m_1 = {}
            o1h, o1r = emit_l1(0, z8m_1)
            for u in range(2):
                drain_h(o1h[u], b1[:, 0:1], u, h2, dve=True)
            emit_l0(1)
            emit_l1_dr(0, o1r, z8m_1)
            for u in range(2):
                drain_r(o1r[u], b1[:, 1:2], u, r1)

            z8m2 = {}
            emit_l2_z(0, z8m2)
            red_flush()   # r0 t2/t3 + r1(0) reduces
            # fb(1) reuses the fbh buffers: emit only after every half-0
            # reader (incl. L2's z muls) is in the graph.
            emit_fb(1)
            o2 = emit_l2_mms(0, z8m2)
            for u in range(2):
                drain_r(o2[0][u], b2[:, 0:1], u, r2a)
                drain_r(o2[1][u], b2[:, 1:2], u, r2b)

            # half 1 (emit_l1's q3 flush emits half-0's r2 reduces; out(0)
            # follows so it reads fully-reduced tiles)
            z8m_1 = {}
            o1h, o1r = emit_l1(1, z8m_1)
            if out_dma_split:
                emit_out(0)
            for u in range(2):
                drain_h(o1h[u], b1[:, 0:1], 2 + u, h2, dve=True)
            emit_l1_dr(1, o1r, z8m_1)
            for u in range(2):
                drain_r(o1r[u], b1[:, 1:2], 2 + u, r1)

            z8m2 = {}
            emit_l2_z(1, z8m2)
            red_flush()
            o2 = emit_l2_mms(1, z8m2)
            for u in range(2):
                drain_r(o2[0][u], b2[:, 0:1], 2 + u, r2a)
                drain_r(o2[1][u], b2[:, 1:2], 2 + u, r2b)
            red_flush()
            if out_dma_split:
                emit_out(1)

            if not out_dma_split:
                nc.sync.dma_start(out_d[0:128, :], r0)
                nc.sync.dma_start(out_d[128:256, :], r1)
                nc.sync.dma_start(out_d[256:384, :], r2a)
                nc.sync.dma_start(out_d[384:512, :], r2b)

    nc.compile()
    return nc


def _host_prep(feat, W0, b0, W1, b1, W2, b2):
    """Rearrange full inputs into the per-core in_maps."""
    import ml_dtypes

    E4 = ml_dtypes.float8_e4m3fn
    E5 = ml_dtypes.float8_e5m2
    feat = np.ascontiguousarray(feat, dtype=np.float32)

    # W0: chunks c of 128 (i,j)-pairs, i-major: p = (i_local, j), i = 4c + p//32
    A = np.ascontiguousarray(W0.transpose(1, 2, 0)).reshape(F0 * F0, K)
    w0t = np.ascontiguousarray(
        A.reshape(NJ0, 128, K).transpose(1, 0, 2).reshape(128, NJ0 * K)
    ).astype(np.float16)

    # L1 kh0 (h-half) fp16: [i, j*128 + m] = W1[m, i, j]
    w1h16 = np.ascontiguousarray(
        W1[:128].transpose(1, 2, 0).reshape(H, F0 * 128)
    ).astype(np.float16)
    # L1 kh1 fp8 1-pass: [i, (g*2+pair)*128 + m] = q8(W1[128+m, i, 2g+pair])
    w1r = W1[128:].transpose(1, 2, 0)            # [i, j, m]
    w1r8 = np.clip(w1r, -240, 240).astype(E4).reshape(H, F0 * 128)
    # fp16 copy of the first j's for the fp16 kh1 quads
    n16_l1 = 4 * (NQ - QF8_L1)
    w1r16 = np.ascontiguousarray(
        w1r[:, : max(n16_l1, 1)].reshape(H, -1)
    ).astype(np.float16)

    # L2 both kh fp8 2-pass: [i, ((g*2+kh)*2+pair)*128 + m] = W2[kh*128+m, i, 2g+pair]
    w2 = W2.transpose(1, 2, 0).reshape(H, NG, 2, 2, 128)  # [i, g, pair, kh, m]
    w2 = np.ascontiguousarray(w2.transpose(0, 1, 3, 2, 4))  # [i, g, kh, pair, m]
    w2hi = np.clip(w2, -240, 240).astype(E4)
    w2lo = (w2 - w2hi.astype(np.float32)).astype(E5)
    w2hi8 = w2hi.reshape(H, NG * 2 * 2 * 128)
    w2lo8 = w2lo.reshape(H, NG * 2 * 2 * 128)

    p_ = np.arange(128)
    s4all = np.zeros((F0, NJ0 * 128), np.float16)
    for cc in range(NJ0):
        s4all[:, cc * 128 : (cc + 1) * 128] = (
            (4 * cc + p_[None, :] // F0) == np.arange(F0)[:, None]
        )

    b0t = np.ascontiguousarray(b0.reshape(2, 128).T).astype(np.float32)
    b1t = np.ascontiguousarray(b1.reshape(2, 128).T).astype(np.float32)
    b2t = np.ascontiguousarray(b2.reshape(2, 128).T).astype(np.float32)

    p = np.arange(128)
    in_maps = []
    for c in range(NCORES):
        fc = feat[c * BL : (c + 1) * BL]                        # [64, 32, 32]
        featT = np.ascontiguousarray(fc.transpose(1, 0, 2)).reshape(F0, BD)
        featT = featT.astype(np.float16)
        featR = np.ascontiguousarray(featT[p % F0])             # [128, BD]
        featH = np.concatenate(
            [
                featT[4 * cc + p // F0, t * NT : (t + 1) * NT]
                for t in range(BD // NT)
                for cc in range(NJ0)
            ],
            axis=1,
        )                                                        # [128, NJ0*BD] t-major
        in_maps.append(
            {
                "featT16": featT,
                "featR": featR,
                "featH": np.ascontiguousarray(featH),
                "s4all": s4all,
                "w0t": w0t,
                "w1h16": w1h16,
                "w1r16": w1r16,
                "w1r8": w1r8.view(np.uint8),
                "w2hi8": w2hi8.view(np.uint8),
                "w2lo8": w2lo8.view(np.uint8),
                "b0t": b0t,
                "b1t": b1t,
                "b2t": b2t,
            }
        )
    return in_maps


def kernel(feat, W0, b0, W1, b1, W2, b2):
    global LAST_RESULTS
    if "nc" not in _CACHE:
        _CACHE["nc"] = _build_program()
    nc = _CACHE["nc"]
    in_maps = _host_prep(feat, W0, b0, W1, b1, W2, b2)
    res = run_bass_kernel_spmd(nc, in_maps, core_ids=list(range(NCORES)))
    LAST_RESULTS = res
    out = np.concatenate([res.results[c]["out"].T for c in range(NCORES)], axis=0)
    return np.ascontiguousarray(out, dtype=np.float32)
